# revision 6
# baseline (speedup 1.0000x reference)
"""LIF (leaky integrate-and-fire) forward recurrence on 8 Trainium2 NeuronCores.

Input  x: (T=16, B=128, N=16384) float32, time-major.
    m[t] = tau * v[t-1] + x[t]
    y[t] = (m[t] >= v_th)            spike, as 0.0/1.0
    v[t] = m[t] * (1 - y[t])         hard reset

Sharding: N is split 8 ways (2048 per core); the recurrence is per-neuron
independent so the cores never communicate.  Per core per timestep the
work is a [128 x 2048] f32 tile:
    m  = (v * tau) + x[t]        scalar_tensor_tensor on DVE
    y  = (m >= 1.0)              tensor_scalar (2x mode) on DVE
    v' = (m < 1.0) * m           scalar_tensor_tensor on DVE
All three are exact in f32 (tau=0.5 multiply is exact, compare outputs are
exactly 0.0/1.0), so the result is bit-identical to the f32 reference.
"""

import numpy as np

import concourse.bass as bass
import concourse.mybir as mybir
from concourse.bass_utils import run_bass_kernel_spmd
from concourse.mybir import AluOpType
from concourse.tile import TileContext

T, B, N = 16, 128, 16384
NCORES = 8
NSH = N // NCORES  # 2048 neurons per core
TAU = 0.5
V_TH = 1.0

_cached_nc = None


def _split_stt_waits(nc):
    """Walrus codegen only supports ONE sync-wait per compute/DMA instruction
    (single wait slot in the EVENTS field); Tile sometimes attaches two.
    Move the extra waits onto same-engine NoOps inserted right before — the
    sequencer executes in program order, so semantics are unchanged."""
    multi_ok = (mybir.InstEventSemaphore, mybir.InstNoOp)
    for f in nc.m.functions:
        for b in f.blocks:
            new_insts = []
            for inst in b.instructions:
                si = inst.sync_info
                if (
                    not isinstance(inst, multi_ok)
                    and si is not None
                    and len(si.on_wait) > 1
                ):
                    waits = list(si.on_wait)
                    for j, w in enumerate(waits[:-1]):
                        new_insts.append(
                            mybir.InstNoOp(
                                name=f"{inst.name}_presync{j}",
                                engine=inst.engine,
                                sync_info=mybir.SyncInfo(on_wait=[w], on_update=[]),
                            )
                        )
                    inst.sync_info = mybir.SyncInfo(
                        on_wait=[waits[-1]], on_update=list(si.on_update)
                    )
                new_insts.append(inst)
            b.instructions = new_insts


def _build():
    nc = bass.Bass(trn_type="TRN2")
    x = nc.dram_tensor("x", [T, B, NSH], mybir.dt.float32, kind="ExternalInput")
    y = nc.dram_tensor("y", [T, B, NSH], mybir.dt.float32, kind="ExternalOutput")

    with TileContext(nc) as tc:
        with (
            tc.tile_pool(name="state", bufs=1) as state_pool,
            tc.tile_pool(name="xin", bufs=4) as xin_pool,
            tc.tile_pool(name="yout", bufs=4) as yout_pool,
            tc.tile_pool(name="work", bufs=2) as work_pool,
        ):
            v = state_pool.tile([B, NSH], mybir.dt.float32)
            nc.vector.memset(v[:], 0.0)
            for t in range(T):
                xt = xin_pool.tile([B, NSH], mybir.dt.float32, tag="xt")
                nc.sync.dma_start(out=xt[:], in_=x[t])
                m = work_pool.tile([B, NSH], mybir.dt.float32, tag="m")
                # m = v * tau + x[t]
                nc.vector.scalar_tensor_tensor(
                    m[:], v[:], TAU, xt[:], AluOpType.mult, AluOpType.add
                )
                yt = yout_pool.tile([B, NSH], mybir.dt.float32, tag="yt")
                # y = (m >= v_th)  -> exactly 0.0 / 1.0
                nc.vector.tensor_scalar(yt[:], m[:], V_TH, None, AluOpType.is_ge)
                # v = (m < v_th) * m   (hard reset)
                nc.vector.scalar_tensor_tensor(
                    v[:], m[:], V_TH, m[:], AluOpType.is_lt, AluOpType.mult
                )
                nc.sync.dma_start(out=y[t], in_=yt[:])
    _split_stt_waits(nc)
    return nc


def kernel(x: np.ndarray) -> np.ndarray:
    global _cached_nc
    if _cached_nc is None:
        _cached_nc = _build()
    nc = _cached_nc

    x = np.ascontiguousarray(x, dtype=np.float32)
    assert x.shape == (T, B, N)
    in_maps = [
        {"x": np.ascontiguousarray(x[:, :, k * NSH : (k + 1) * NSH])}
        for k in range(NCORES)
    ]
    res = run_bass_kernel_spmd(nc, in_maps, core_ids=list(range(NCORES)))
    global _last_exec_ns
    if res.exec_time_ns is not None:
        _last_exec_ns = res.exec_time_ns
    return np.concatenate([r["y"] for r in res.results], axis=2)


_last_exec_ns = None
